# revision 2
# baseline (speedup 1.0000x reference)
"""Global-average-pool + sigmoid channel scores on 8 trn2 NeuronCores — v2.

Raw-bass (no TileContext) rewrite of the baseline.  Same data-parallel
sharding (core i reduces contiguous batch shard x[4i:4i+4], viewed as
[256, 50176]), but the cross-core combine is hand-rolled with
remote_dma_broadcast instead of an ncfw AllGather:

  * ncfw collectives on this stack cost ~19-29us each plus a one-time ~47us
    alignment barrier (absorbed by a warm-up collective in the baseline).
    The baseline tail after streaming was ~53us.
  * Here every core posts its [128,1] partial-sum column into the seven
    peers' SBUF `gath` slots via seven single-dest relative
    remote_dma_broadcast descriptors (descgen runs at t~0 on the SWDGE ring,
    off the critical path), fired by one trigger_dma when the local partial
    is ready.  Cross-core sync is the remote semaphore: each transfer
    increments the receiver's rx sem by 2 (2 SDMA lanes); waiting for 14
    means all seven peers' columns have landed.
  * Relative dests (delta-tpb XOR) make the program SPMD-symmetric: no
    runtime routing registers, no host topology queries.  8 cores = 8
    physical NCs of one chip (LNC1, trn2.8x1), so delta-tpb 1..7 with
    cross-die deltas (bit 2 set) in slots 4-7 is legal routing.

Streaming is a hand-rolled 6-deep pipeline on the sync (HWDGE) queue with a
tapered tail (last pieces 3136/1568/784/784 wide) so the final vector reduce
drains ~1us after the last byte instead of ~6us.

v4: the cross-core combine is the ncfw AllGather (as in the tile baseline).
A hand-rolled remote_dma all-to-all was measured slower here: a [128,1]
SBUF->SBUF remote transfer is 128 four-byte fabric packets at ~55-75ns
each, and the seven transfers serialize per link (~40-55us total), while
the ncfw AG completes in ~19-28us after the last rank's trigger.
"""

import os

import numpy as np

try:
    import concourse.bass as bass  # noqa: F401
except ImportError:  # pragma: no cover - fallback when site path is absent
    import sys

    for p in ("/opt/trn_rl_repo", "/root/.axon_site/_ro/trn_rl_repo"):
        if p not in sys.path:
            sys.path.insert(0, p)

import concourse.bacc as bacc
import concourse.mybir as mybir
from concourse.bass_utils import run_bass_kernel_spmd

N_CORES = 8
B, C, H, W = 32, 64, 224, 224
B_LOC = B // N_CORES            # 4 batches per core
ROWS = B_LOC * C                # 256 (b_loc, c) rows per core
HW = H * W                      # 50176 spatial elements per row
N_PTILES = ROWS // 128          # 2 partition tiles of 128 rows
CHUNK = 6272                    # full-width piece: 128 x 6272 f32 = 3.2 MB
MEAN_SCALE = 1.0 / (B * HW)
N_BUF = 6                       # streaming double-buffer depth

# Piece widths per partition tile.  ptile 0 streams at full width; ptile 1
# tapers so the last reduces drain quickly behind the final DMAs.
_TAPER = [3136, 1568, 784, 784]
assert sum(_TAPER) == CHUNK
WIDTHS_P0 = [CHUNK] * 8
WIDTHS_P1 = [CHUNK] * 7 + _TAPER
assert sum(WIDTHS_P0) == HW and sum(WIDTHS_P1) == HW

_CACHE = {}


def _build():
    nc = bacc.Bacc(
        "TRN2",
        target_bir_lowering=False,
        debug=False,
        num_devices=N_CORES,
    )
    f32 = mybir.dt.float32
    xs = nc.dram_tensor("xs", [ROWS, HW], f32, kind="ExternalInput")
    out = nc.dram_tensor("out", [B, C], f32, kind="ExternalOutput")
    xs_ap = xs.ap()
    out_ap = out.ap()

    pieces = []  # (ptile, col_start, width)
    for n, widths in enumerate((WIDTHS_P0, WIDTHS_P1)):
        col = 0
        for w in widths:
            pieces.append((n, col, w))
            col += w
    n_pieces = len(pieces)

    from contextlib import ExitStack

    st = ExitStack()
    buf = [
        st.enter_context(nc.sbuf_tensor(f"buf{k}", [128, CHUNK], f32))
        for k in range(N_BUF)
    ]
    stats = st.enter_context(nc.sbuf_tensor("stats", [128, n_pieces], f32))
    gath = st.enter_context(nc.sbuf_tensor("gath", [128, N_CORES], f32))
    total = st.enter_context(nc.sbuf_tensor("total", [128, 1], f32))
    row = st.enter_context(nc.sbuf_tensor("row", [1, N_CORES * 128], f32))
    pre = st.enter_context(nc.sbuf_tensor("pre", [1, C], f32))
    scores = st.enter_context(nc.sbuf_tensor("scores", [1, C], f32))
    rep = st.enter_context(nc.sbuf_tensor("rep", [B, C], f32))
    ones = st.enter_context(nc.sbuf_tensor("ones", [1, B], f32))
    psr = st.enter_context(nc.psum_tensor("psr", [B, C], f32))

    # one DMA sem per buffer slot (rotating, tile-DMAHW style: consecutive
    # +16s on a single sem from FIFO DMAs trip the sim's sem-race rule)
    s_dma = [nc.alloc_semaphore(f"s_dma{k}") for k in range(N_BUF)]
    s_vec = nc.alloc_semaphore("s_vec")    # per-piece reduce retirements (+1)
    s_prep = nc.alloc_semaphore("s_prep")  # SWDGE descgen commits (+1)
    s_x = nc.alloc_semaphore("s_x")        # local partial visible for SDMA
    s_ep = nc.alloc_semaphore("s_ep")      # epilogue chain
    s_tp = nc.alloc_semaphore("s_tp")      # SWDGE transpose hop (must start 0)
    s_out = nc.alloc_semaphore("s_out")    # final out DMA
    s_warm = nc.alloc_semaphore("s_warm")  # warm-up collective input staged
    s_cc = nc.alloc_semaphore("s_cc")      # warm-up collective completion
    s_wz = nc.alloc_semaphore("s_wz")      # wz memset drained
    s_one = nc.alloc_semaphore("s_one")    # ones row visible for PE

    s_ag = nc.alloc_semaphore("s_ag")      # final AllGather completion
    s_ld = nc.alloc_semaphore("s_ld")      # cc_out row reload
    my_sems = s_dma + [s_vec, s_prep] + [
        s_x, s_ep, s_tp, s_out, s_warm, s_cc, s_wz, s_one, s_ag, s_ld,
    ]

    # --- warm-up ncfw collective (t~0) ------------------------------------
    # A NEFF with no collectives is dispatched to the 8 cores with multi-ms
    # skew on this stack; any ncfw collective forces the runtime's
    # cross-core rendezvous so execution starts aligned (and absorbs the
    # one-time ~47us CC alignment barrier while the stream is young).
    # Nothing consumes warm_out; the AG runs async on the CC stream.
    warm_in = nc.dram_tensor("warm_in", [1, 1], f32)
    warm_out = nc.dram_tensor("warm_out", [N_CORES, 1], f32, addr_space="Shared")
    wz = st.enter_context(nc.sbuf_tensor("wz", [1, 1], f32))
    nc.gpsimd.memset(wz[:, :], 0.0)
    nc.gpsimd.drain().then_inc(s_wz, 1)
    nc.gpsimd.wait_ge(s_wz, 1)
    nc.gpsimd.dma_start(out=warm_in.ap()[:, :], in_=wz[:, :]).then_inc(s_warm, 16)
    nc.gpsimd.wait_ge(s_warm, 16)
    nc.gpsimd.collective_compute(
        "AllGather",
        mybir.AluOpType.bypass,
        replica_groups=[list(range(N_CORES))],
        ins=[warm_in.ap()[:, :].opt()],
        outs=[warm_out.ap()[:, :].opt()],
    ).then_inc(s_cc, 1)

    # DRAM tiles for the final AllGather (bounce in local, gathered shared)
    cc_in = nc.dram_tensor("cc_in", [128, 1], f32)
    cc_out = nc.dram_tensor("cc_out", [1, N_CORES * 128], f32, addr_space="Shared")

    # ones row for the PE broadcast (vector, t~0; s_one proves visibility)
    nc.vector.memset(ones[:, :], 1.0)
    nc.vector.drain().then_inc(s_one, 1)

    # --- streaming: 6-deep pipelined loads + per-piece reduces ------------
    for i, (n, col, w) in enumerate(pieces):
        if i >= N_BUF:
            nc.sync.wait_ge(s_vec, i - N_BUF + 1)
        nc.sync.dma_start(
            out=buf[i % N_BUF][:, 0:w],
            in_=xs_ap[n * 128 : (n + 1) * 128, col : col + w],
        ).then_inc(s_dma[i % N_BUF], 16)

    for i, (n, col, w) in enumerate(pieces):
        nc.vector.wait_ge(s_dma[i % N_BUF], 16 * (i // N_BUF + 1))
        nc.vector.reduce_sum(
            out=stats[:, i : i + 1],
            in_=buf[i % N_BUF][:, 0:w],
            axis=mybir.AxisListType.X,
        ).then_inc(s_vec, 1)

    # local partial sums (partition p accumulates batches {p//64, p//64+2}
    # of channel p%64)
    nc.vector.wait_ge(s_vec, n_pieces)  # all stats columns written+visible
    nc.vector.reduce_sum(
        out=total[:, :], in_=stats[:, 0:n_pieces], axis=mybir.AxisListType.X
    )
    nc.vector.drain().then_inc(s_x, 1)

    # --- bounce to DRAM (HWDGE, fast) and fire the AllGather --------------
    nc.sync.wait_ge(s_x, 1)
    nc.sync.dma_start(out=cc_in.ap()[:, :], in_=total[:, :]).then_inc(s_tp, 16)
    nc.gpsimd.wait_ge(s_tp, 16)
    nc.gpsimd.collective_compute(
        "AllGather",
        mybir.AluOpType.bypass,
        replica_groups=[list(range(N_CORES))],
        ins=[cc_in.ap()[:, :].opt()],
        outs=[cc_out.ap()[:, :].opt()],
    ).then_inc(s_ag, 1)

    # --- reload all ranks' partials as one row and fold -------------------
    nc.sync.wait_ge(s_ag, 1)
    nc.sync.dma_start(out=row[0:1, :], in_=cc_out.ap()[:, :]).then_inc(s_ld, 16)

    # fold ranks AND the two batch-parity halves in one strided reduce:
    # element (r, b, c) sits at 128r + 64b + c
    nc.vector.wait_ge(s_ld, 16)
    nc.vector.reduce_sum(
        out=pre[:, :],
        in_=row[:, :].rearrange("o (r b c) -> o c (r b)", r=N_CORES, b=2),
        axis=mybir.AxisListType.X,
    )
    nc.vector.drain().then_inc(s_ep, 1)

    nc.scalar.wait_ge(s_ep, 1)
    nc.scalar.activation(
        scores[:, :],
        pre[:, :],
        mybir.ActivationFunctionType.Sigmoid,
        scale=MEAN_SCALE,
    )
    nc.scalar.drain().then_inc(s_ep, 1)

    nc.tensor.wait_ge(s_one, 1)
    nc.tensor.wait_ge(s_ep, 2)
    nc.tensor.matmul(psr[:, :], ones[:, :], scores[:, :])
    nc.tensor.drain().then_inc(s_ep, 1)

    nc.vector.wait_ge(s_ep, 3)
    nc.vector.tensor_copy(rep[:, :], psr[:, :])
    nc.vector.drain().then_inc(s_ep, 1)

    nc.sync.wait_ge(s_ep, 4)
    nc.sync.dma_start(out=out_ap[:, :], in_=rep[:, :]).then_inc(s_out, 16)

    # --- postamble: leave every semaphore at 0 for the next execution -----
    nc.gpsimd.wait_ge(s_out, 16)             # out DMA landed
    nc.gpsimd.wait_ge(s_ag, 1)
    for k in range(N_BUF):                   # all streaming DMAs retired
        nc.gpsimd.wait_ge(s_dma[k], 16 * len(range(k, n_pieces, N_BUF)))
    nc.gpsimd.wait_ge(s_vec, n_pieces)
    nc.gpsimd.wait_ge(s_cc, 1)               # warm-up AG long since done
    nc.all_engine_barrier()                  # registered barrier: RD prunes
    nums = sorted(s.num for s in my_sems)
    runs = []
    for n in nums:
        if runs and n == runs[-1][1]:
            runs[-1][1] = n + 1
        else:
            runs.append([n, n + 1])
    for a, b in runs:
        nc.gpsimd.dma_reset(range(a, b))
        nc.gpsimd.sem_clear(range(a, b))

    nc.compile()
    st.close()
    return nc


def _get_nc():
    if "nc" not in _CACHE:
        _CACHE["nc"] = _build()
    return _CACHE["nc"]


def _in_maps(x: np.ndarray):
    x = np.ascontiguousarray(np.asarray(x, dtype=np.float32))
    return [
        {"xs": x[i * B_LOC : (i + 1) * B_LOC].reshape(ROWS, HW)}
        for i in range(N_CORES)
    ]


def _run(x: np.ndarray, **kwargs):
    return run_bass_kernel_spmd(_get_nc(), _in_maps(x), list(range(N_CORES)), **kwargs)


def kernel(x: np.ndarray) -> np.ndarray:
    res = _run(x)
    return np.asarray(res.results[0]["out"], dtype=np.float32)


# revision 3
# speedup vs baseline: 1.3150x; 1.3150x over previous
"""Global-average-pool + sigmoid channel scores on 8 trn2 NeuronCores.

Problem: x (32, 64, 224, 224) f32 -> sigmoid(mean(x, axes=(0,2,3))) broadcast
to (32, 64).  Data-parallel over batch: core i reduces the contiguous shard
x[4i:4i+4], cores AllGather per-partition partial sums, and each core
finishes the cross-core/cross-batch folds + sigmoid + broadcast locally
(output replicated; host takes core 0's copy).

Collective cost on this stack (measured over many runs): each collective
costs 20-45us regardless of payload, throttles streaming DMA while active,
and is only cheap when chained immediately behind another collective.  The
net-optimal structure is therefore: one 4-byte warm-up AllGather at t=0
(absorbs the cross-core alignment barrier + ncfw first-call cost while the
stream is young), a completely quiet CC stream for the rest of the
streaming phase, and a single real AllGather at the end.
"""

import numpy as np

try:
    import concourse.bass as bass  # noqa: F401
except ImportError:  # pragma: no cover - fallback when site path is absent
    import sys

    for p in ("/opt/trn_rl_repo", "/root/.axon_site/_ro/trn_rl_repo"):
        if p not in sys.path:
            sys.path.insert(0, p)

import concourse.bass as bass
import concourse.bacc as bacc
import concourse.mybir as mybir
import concourse.tile as tile
from concourse.bass_utils import run_bass_kernel_spmd

N_CORES = 8
B, C, H, W = 32, 64, 224, 224
B_LOC = B // N_CORES            # 4 batches per core
ROWS = B_LOC * C                # 256 (b_loc, c) rows per core
HW = H * W                      # 50176 spatial elements per row
N_PTILES = ROWS // 128          # 2 partition tiles of 128 rows
CHUNK = 6272                    # 50176 = 8 * 6272; 3.2 MB per DMA tile
N_CHUNKS = HW // CHUNK          # 8 free-dim chunks per partition tile
MEAN_SCALE = 1.0 / (B * HW)     # mean over batch+spatial = 32*50176 elems
TAPER = [3136, 1568, 784, 784]  # final chunk split so reduces drain fast

_CACHE = {}


def _build():
    nc = bacc.Bacc(
        "TRN2",
        target_bir_lowering=False,
        debug=False,
        num_devices=N_CORES,
    )
    xs = nc.dram_tensor("xs", [ROWS, HW], mybir.dt.float32, kind="ExternalInput")
    out = nc.dram_tensor("out", [B, C], mybir.dt.float32, kind="ExternalOutput")
    xs_ap = xs.ap()
    out_ap = out.ap()
    rg = [list(range(N_CORES))]

    pieces = []  # (row_tile_idx, col_start, width)
    for n in range(N_PTILES):
        for j in range(N_CHUNKS):
            if n == N_PTILES - 1 and j == N_CHUNKS - 1:
                col = j * CHUNK
                for w in TAPER:
                    pieces.append((n, col, w))
                    col += w
            else:
                pieces.append((n, j * CHUNK, CHUNK))
    n_pieces = len(pieces)

    with tile.TileContext(nc) as tc:
        with (
            tc.tile_pool(name="data", bufs=6) as data_pool,
            tc.tile_pool(name="small", bufs=1) as small_pool,
            tc.tile_pool(name="dram", bufs=1, space="DRAM") as dram_pool,
        ):
            # First warm-up collective, entirely on gpsimd so it fires
            # immediately after the kernel preamble.
            warm_in = dram_pool.tile([1, 1], mybir.dt.float32)
            warm_out = dram_pool.tile([N_CORES, 1], mybir.dt.float32)
            wz = small_pool.tile([1, 1], mybir.dt.float32)
            nc.gpsimd.memset(wz[:, :], 0.0)
            nc.gpsimd.dma_start(out=warm_in[:, :], in_=wz[:, :])
            nc.gpsimd.collective_compute(
                "AllGather",
                mybir.AluOpType.bypass,
                replica_groups=rg,
                ins=[warm_in[:, :].opt()],
                outs=[warm_out[:, :].opt()],
            )

            stats = small_pool.tile([128, n_pieces], mybir.dt.float32)
            for i, (n, col, width) in enumerate(pieces):
                t_in = data_pool.tile([128, width], mybir.dt.float32, tag="data")
                nc.sync.dma_start(
                    out=t_in[:, 0:width],
                    in_=xs_ap[n * 128 : (n + 1) * 128, col : col + width],
                )
                nc.vector.reduce_sum(
                    out=stats[:, i : i + 1],
                    in_=t_in[:, 0:width],
                    axis=mybir.AxisListType.X,
                )


            # Final collective over all pieces.  Bounce DMA via gpsimd SWDGE
            # after streaming has drained, so the HWDGE rings never stall.
            psum = small_pool.tile([128, 1], mybir.dt.float32)
            nc.vector.reduce_sum(
                out=psum[:, :], in_=stats[:, 0:n_pieces], axis=mybir.AxisListType.X
            )
            cc_in = dram_pool.tile([128, 1], mybir.dt.float32)
            cc_out = dram_pool.tile([1, N_CORES * 128], mybir.dt.float32)
            nc.sync.dma_start(out=cc_in[:, :], in_=psum[:, :])
            nc.gpsimd.collective_compute(
                "AllGather",
                mybir.AluOpType.bypass,
                replica_groups=rg,
                ins=[cc_in[:, :].opt()],
                outs=[cc_out[:, :].opt()],
            )

            # All 8 ranks' partials live contiguously (rank-major); reload on
            # one partition, then halve 4 times: 1024 -> 512 -> 256 -> 128
            # folds ranks, 128 -> 64 folds the two batch halves, leaving
            # per-channel totals.
            row = small_pool.tile([1, N_CORES * 128], mybir.dt.float32)
            nc.sync.dma_start(out=row[:, :], in_=cc_out[:, :])

            # Fold ranks AND the two batch halves with one strided reduce:
            # element (r, b, c) sits at 128r + 64b + c, so viewing the row as
            # [c, (r b)] puts all 16 contributions of channel c on the X axis.
            folded = small_pool.tile([1, C], mybir.dt.float32)
            nc.vector.reduce_sum(
                out=folded[:, :],
                in_=row[:, :].rearrange("o (r b c) -> o c (r b)", r=N_CORES, b=2),
                axis=mybir.AxisListType.X,
            )

            scores = small_pool.tile([1, C], mybir.dt.float32)
            nc.scalar.activation(
                scores[:, :],
                folded[:, :],
                mybir.ActivationFunctionType.Sigmoid,
                scale=MEAN_SCALE,
            )

            rep = small_pool.tile([B, C], mybir.dt.float32)
            nc.gpsimd.partition_broadcast(rep[:, :], scores[:, :])
            nc.sync.dma_start(out=out_ap[:, :], in_=rep[:, :])

    nc.compile()
    return nc


def _get_nc():
    if "nc" not in _CACHE:
        _CACHE["nc"] = _build()
    return _CACHE["nc"]


def _in_maps(x: np.ndarray):
    x = np.ascontiguousarray(np.asarray(x, dtype=np.float32))
    return [
        {"xs": x[i * B_LOC : (i + 1) * B_LOC].reshape(ROWS, HW)}
        for i in range(N_CORES)
    ]


def _run(x: np.ndarray, **kwargs):
    return run_bass_kernel_spmd(_get_nc(), _in_maps(x), list(range(N_CORES)), **kwargs)


def kernel(x: np.ndarray) -> np.ndarray:
    res = _run(x)
    return np.asarray(res.results[0]["out"], dtype=np.float32)



# revision 4
# speedup vs baseline: 1.3890x; 1.0562x over previous
"""Global-average-pool + sigmoid channel scores on 8 trn2 NeuronCores.

Problem: x (32, 64, 224, 224) f32 -> sigmoid(mean(x, axes=(0,2,3))) broadcast
to (32, 64).  Data-parallel over batch: core i reduces the contiguous shard
x[4i:4i+4], cores AllGather per-partition partial sums, and each core
finishes the cross-core/cross-batch folds + sigmoid + broadcast locally
(output replicated; host takes core 0's copy).

Collective cost on this stack (measured over many runs): each collective
costs 20-45us regardless of payload, throttles streaming DMA while active,
and is only cheap when chained immediately behind another collective.  The
net-optimal structure is therefore: one 4-byte warm-up AllGather at t=0
(absorbs the cross-core alignment barrier + ncfw first-call cost while the
stream is young), a completely quiet CC stream for the rest of the
streaming phase, and a single real AllGather at the end.
"""

import numpy as np

try:
    import concourse.bass as bass  # noqa: F401
except ImportError:  # pragma: no cover - fallback when site path is absent
    import sys

    for p in ("/opt/trn_rl_repo", "/root/.axon_site/_ro/trn_rl_repo"):
        if p not in sys.path:
            sys.path.insert(0, p)

import concourse.bass as bass
import concourse.bacc as bacc
import concourse.mybir as mybir
import concourse.tile as tile
from concourse.bass_utils import run_bass_kernel_spmd

N_CORES = 8
B, C, H, W = 32, 64, 224, 224
B_LOC = B // N_CORES            # 4 batches per core
ROWS = B_LOC * C                # 256 (b_loc, c) rows per core
HW = H * W                      # 50176 spatial elements per row
N_PTILES = ROWS // 128          # 2 partition tiles of 128 rows
CHUNK = 6272                    # 50176 = 8 * 6272; 3.2 MB per DMA tile
N_CHUNKS = HW // CHUNK          # 8 free-dim chunks per partition tile
MEAN_SCALE = 1.0 / (B * HW)     # mean over batch+spatial = 32*50176 elems
TAPER = [3136, 1568, 784, 784]  # final chunk split so reduces drain fast

_CACHE = {}


def _build():
    nc = bacc.Bacc(
        "TRN2",
        target_bir_lowering=False,
        debug=False,
        num_devices=N_CORES,
    )
    xs = nc.dram_tensor("xs", [ROWS, HW], mybir.dt.float32, kind="ExternalInput")
    out = nc.dram_tensor("out", [B, C], mybir.dt.float32, kind="ExternalOutput")
    xs_ap = xs.ap()
    out_ap = out.ap()
    rg = [list(range(N_CORES))]

    pieces = []  # (row_tile_idx, col_start, width)
    for n in range(N_PTILES):
        for j in range(N_CHUNKS):
            if n == N_PTILES - 1 and j == N_CHUNKS - 1:
                col = j * CHUNK
                for w in TAPER:
                    pieces.append((n, col, w))
                    col += w
            else:
                pieces.append((n, j * CHUNK, CHUNK))
    n_pieces = len(pieces)

    with tile.TileContext(nc) as tc:
        with (
            tc.tile_pool(name="data", bufs=6) as data_pool,
            tc.tile_pool(name="small", bufs=1) as small_pool,
            tc.tile_pool(name="dram", bufs=1, space="DRAM") as dram_pool,
        ):
            # First warm-up collective, entirely on gpsimd so it fires
            # immediately after the kernel preamble.
            warm_in = dram_pool.tile([1, 1], mybir.dt.float32)
            warm_out = dram_pool.tile([N_CORES, 1], mybir.dt.float32)
            wz = small_pool.tile([1, 1], mybir.dt.float32)
            nc.gpsimd.memset(wz[:, :], 0.0)
            nc.gpsimd.dma_start(out=warm_in[:, :], in_=wz[:, :])
            nc.gpsimd.collective_compute(
                "AllGather",
                mybir.AluOpType.bypass,
                replica_groups=rg,
                ins=[warm_in[:, :].opt()],
                outs=[warm_out[:, :].opt()],
            )

            stats = small_pool.tile([128, n_pieces], mybir.dt.float32)
            for i, (n, col, width) in enumerate(pieces):
                t_in = data_pool.tile([128, width], mybir.dt.float32, tag="data")
                nc.sync.dma_start(
                    out=t_in[:, 0:width],
                    in_=xs_ap[n * 128 : (n + 1) * 128, col : col + width],
                )
                nc.vector.reduce_sum(
                    out=stats[:, i : i + 1],
                    in_=t_in[:, 0:width],
                    axis=mybir.AxisListType.X,
                )


            # Final collective over all pieces.  Bounce DMA via gpsimd SWDGE
            # after streaming has drained, so the HWDGE rings never stall.
            psum = small_pool.tile([128, 1], mybir.dt.float32)
            nc.vector.reduce_sum(
                out=psum[:, :], in_=stats[:, 0:n_pieces], axis=mybir.AxisListType.X
            )
            cc_in = dram_pool.tile([128, 1], mybir.dt.float32)
            cc_out = dram_pool.tile([1, N_CORES * 128], mybir.dt.float32)
            nc.gpsimd.dma_start(out=cc_in[:, :], in_=psum[:, :])
            nc.gpsimd.collective_compute(
                "AllGather",
                mybir.AluOpType.bypass,
                replica_groups=rg,
                ins=[cc_in[:, :].opt()],
                outs=[cc_out[:, :].opt()],
            )

            # All 8 ranks' partials live contiguously (rank-major); reload on
            # one partition, then halve 4 times: 1024 -> 512 -> 256 -> 128
            # folds ranks, 128 -> 64 folds the two batch halves, leaving
            # per-channel totals.
            row = small_pool.tile([1, N_CORES * 128], mybir.dt.float32)
            nc.sync.dma_start(out=row[:, :], in_=cc_out[:, :])

            # Fold ranks AND the two batch halves with one strided reduce:
            # element (r, b, c) sits at 128r + 64b + c, so viewing the row as
            # [c, (r b)] puts all 16 contributions of channel c on the X axis.
            folded = small_pool.tile([1, C], mybir.dt.float32)
            nc.vector.reduce_sum(
                out=folded[:, :],
                in_=row[:, :].rearrange("o (r b c) -> o c (r b)", r=N_CORES, b=2),
                axis=mybir.AxisListType.X,
            )

            scores = small_pool.tile([1, C], mybir.dt.float32)
            nc.scalar.activation(
                scores[:, :],
                folded[:, :],
                mybir.ActivationFunctionType.Sigmoid,
                scale=MEAN_SCALE,
            )

            rep = small_pool.tile([B, C], mybir.dt.float32)
            nc.gpsimd.partition_broadcast(rep[:, :], scores[:, :])
            nc.sync.dma_start(out=out_ap[:, :], in_=rep[:, :])

    nc.compile()
    return nc


def _get_nc():
    if "nc" not in _CACHE:
        _CACHE["nc"] = _build()
    return _CACHE["nc"]


def _in_maps(x: np.ndarray):
    x = np.ascontiguousarray(np.asarray(x, dtype=np.float32))
    return [
        {"xs": x[i * B_LOC : (i + 1) * B_LOC].reshape(ROWS, HW)}
        for i in range(N_CORES)
    ]


def _run(x: np.ndarray, **kwargs):
    return run_bass_kernel_spmd(_get_nc(), _in_maps(x), list(range(N_CORES)), **kwargs)


def kernel(x: np.ndarray) -> np.ndarray:
    res = _run(x)
    return np.asarray(res.results[0]["out"], dtype=np.float32)

